# revision 37
# baseline (speedup 1.0000x reference)
"""Trainium2 Bass kernel for GQA attention (32 q heads / 16 kv heads, head_dim
128, L=2048, D=4608) with RoPE, tanh softcap 50, causal mask, o_proj.

Strategy: tensor-parallel over heads across 8 NeuronCores. Core c computes
q-heads 4c..4c+3 and kv-heads 2c..2c+1 end-to-end (QKV projections, RoPE,
softcapped causal attention, and the partial o_proj against its 512 columns of
wo); the host sums the 8 partial [L, D] outputs.

Per-core layout choices:
  - host passes x, weights pre-transposed (xT [D,L], wqT/wkT/wvT [D,*],
    woT [512,D]) and bf16-cast; all matmuls run bf16 with fp32 PSUM accumulation
  - projections produce QT/KT/VT in [head_dim, L] layout directly
  - RoPE applied in [d, l] layout via host cos/sin tables ([128, L]; sin table
    carries the rotate-half sign split); the 1/sqrt(144) q-scale is folded into
    the softcap activation scale
  - scores computed transposed, ST[k, q] = (KT tile)^T @ QT chunk, so the
    exp(softcap) output PT[k, q] feeds the PV matmul as lhsT with no transpose
  - softcap bounds scores to +-50 so softmax needs no max-subtraction:
    p = exp(50*tanh(s/600)), denominator = sum_k p obtained free via a
    ones-column appended to the V rhs tile
  - attention output [q, d] is divided by the denominator, PE-transposed to
    [d, q], and accumulated into the o_proj over the 4 local heads
"""

import os

import numpy as np
import ml_dtypes

import concourse.bass as bass
import concourse.mybir as mybir
import concourse.tile as tile
from concourse.masks import make_identity
from concourse import bacc

F32 = mybir.dt.float32
BF16 = mybir.dt.bfloat16
BF16_NP = ml_dtypes.bfloat16
AF = mybir.ActivationFunctionType

N_HEADS = 32
N_KV = 16
HEAD_DIM = 128
ROPE_THETA = 10000.0
SOFTCAP = 50.0
SCALE = 1.0 / 12.0  # 1/sqrt(144)
L = 2048
D = 4608
N_CORES = 8
QH = N_HEADS // N_CORES        # 4 local q heads
KVH = N_KV // N_CORES          # 2 local kv heads
KC = D // 128                  # 36 contraction chunks
NQ = L // 512                  # 4 l-chunks of 512
LT = L // 128                  # 16 l-tiles of 128
DOUT_CHUNKS = D // 512         # 9 o_proj output chunks


def _emit(nc, n_iters=1):
    # packed layouts: k-chunks concatenated along the free dim so DMA lines
    # are multi-KB and descriptor counts are small (the DMA fabric moves
    # ~1KB-line tiles at only ~165 GB/s, descriptor issue ~0.6us each)
    xp_d = nc.dram_tensor("xp", [NQ, 128, KC * 512], BF16, kind="ExternalInput")
    wqp_d = nc.dram_tensor("wqp", [128, KC * 512], BF16, kind="ExternalInput")
    wkp_d = nc.dram_tensor("wkp", [128, KC * 256], BF16, kind="ExternalInput")
    wvp_d = nc.dram_tensor("wvp", [128, KC * 256], BF16, kind="ExternalInput")
    wop_d = nc.dram_tensor("wop", [128, QH * D], BF16, kind="ExternalInput")
    cost_d = nc.dram_tensor("cost", [128, L], BF16, kind="ExternalInput")
    sint_d = nc.dram_tensor("sint", [128, L], BF16, kind="ExternalInput")
    masks_d = nc.dram_tensor("masks", [4, 128, 512], BF16, kind="ExternalInput")
    out_d = nc.dram_tensor("out", [L, D], BF16, kind="ExternalOutput")

    with tile.TileContext(nc) as tc:
        with (
            tc.tile_pool(name="const", bufs=1) as const,
            tc.tile_pool(name="persist", bufs=1) as persist,
        ):
            ident = const.tile([128, 128], BF16)
            make_identity(nc, ident[:])
            cost = const.tile([128, L], BF16)
            sint = const.tile([128, L], BF16)
            maskt = [const.tile([128, 512], BF16, tag=f"mask{o}", name=f"mask{o}")
                     for o in range(4)]

            def load_consts():
                # issued after the prologue weight/x DMAs — first needed at
                # the first rope drain, ~50us in
                nc.sync.dma_start(cost[:], cost_d[:])
                nc.sync.dma_start(sint[:], sint_d[:])
                for o in range(4):
                    nc.sync.dma_start(maskt[o][:], masks_d[o])

            # persistent per-head tensors
            QT = [persist.tile([128, L], BF16, tag=f"qt{h}", name=f"qt{h}") for h in range(QH)]
            KT = [persist.tile([128, L], BF16, tag=f"kt{g}", name=f"kt{g}") for g in range(KVH)]
            # V extended with a ones column per k-tile: [128, 16*129]
            VE = [persist.tile([128, LT * 129], BF16, tag=f"ve{g}", name=f"ve{g}") for g in range(KVH)]

            def body():
                _phases(nc, tc, ident, cost, sint, maskt, QT, KT, VE,
                        xp_d, wqp_d, wkp_d, wvp_d, wop_d, out_d, load_consts)

            if n_iters == 1:
                body()
            else:
                with tc.For_i(0, n_iters, 1,
                              hint_engines=(mybir.EngineType.PE,
                                            mybir.EngineType.Activation,
                                            mybir.EngineType.DVE,
                                            mybir.EngineType.SP)):
                    body()
    return nc


def _phases(nc, tc, ident, cost, sint, maskt, QT, KT, VE,
            xp_d, wqp_d, wkp_d, wvp_d, wop_d, out_d, load_consts):
            # ---------------- phase 1: projections + rope ----------------
            MUL = mybir.AluOpType.mult

            def drain_rope(ps, dst, nq, fold):
                """psum [128,512] f32 -> rope -> dst bf16 [128,512] slice.

                Entirely on DVE (fused copy*table via scalar_tensor_tensor)
                so the ACT engine stays free — at the last chunk this lets
                the phase-2 tanh/exp stream start immediately."""
                cols = slice(nq * 512, (nq + 1) * 512)
                raw = rtmp.tile([128, 512], F32, tag="raw")
                nc.vector.scalar_tensor_tensor(
                    raw[:], ps[:], 1.0, cost[:, cols], MUL, MUL)
                swap = rtmp.tile([128, 512], F32, tag="swap")
                nc.vector.scalar_tensor_tensor(
                    swap[0:64, :], ps[64:128, :], 1.0, sint[0:64, cols], MUL, MUL)
                nc.vector.scalar_tensor_tensor(
                    swap[64:128, :], ps[0:64, :], 1.0, sint[64:128, cols], MUL, MUL)
                nc.vector.tensor_add(dst[:, cols], raw[:], swap[:])

            GK = 6           # k-chunks per DMA part
            NPART = KC // GK  # 6 parts per packed tensor
            with (
                tc.tile_pool(name="xcol", bufs=2) as xcol,
                tc.tile_pool(name="rtmp", bufs=3) as rtmp,
                tc.tile_pool(name="wts", bufs=1) as wts,
                tc.tile_pool(name="qk_psum", bufs=1, space="PSUM") as qk_psum,
                tc.tile_pool(name="v_psum", bufs=2, space="PSUM") as v_psum,
            ):
                # k-streamed projections: per 128-wide x-column chunk k, run
                # the 4 Q chains + 2 K chains (6 persistent PSUM banks), so
                # the PE paces just behind the part-granular DMA stream
                # (wq parts on the sync queue; x/wk parts on the scalar
                # queue). V (x stationary as lhsT) runs after each nq's QK
                # block, while the rope drains free the QK banks.
                wqp = wts.tile([128, KC * 512], BF16, tag="wqp", name="wqp")
                wkp = wts.tile([128, KC * 256], BF16, tag="wkp", name="wkp")
                wvp = wts.tile([128, KC * 256], BF16, tag="wvp", name="wvp")
                xc0 = xcol.tile([128, KC * 512], BF16, tag="xp", name="xc0")
                bounds = [0, 1, 2, 4, 6, 12, 18, 24, 30, 36]  # small parts first
                for k0, k1 in zip(bounds, bounds[1:]):
                    q5 = slice(k0 * 512, k1 * 512)
                    q2 = slice(k0 * 256, k1 * 256)
                    nc.sync.dma_start(wqp[:, q5], wqp_d[:, q5])
                    nc.scalar.dma_start(xc0[:, q5], xp_d[0][:, q5])
                    nc.scalar.dma_start(wkp[:, q2], wkp_d[:, q2])
                    nc.sync.dma_start(wvp[:, q2], wvp_d[:, q2])
                load_consts()

                for nq in range(NQ):
                    if nq == 0:
                        xc = xc0
                    else:
                        xc = xcol.tile([128, KC * 512], BF16, tag="xp", name=f"xc{nq}")
                        for p in range(NPART):
                            q5 = slice(p * GK * 512, (p + 1) * GK * 512)
                            nc.sync.dma_start(xc[:, q5], xp_d[nq][:, q5])
                    qps = [qk_psum.tile([128, 512], F32, tag=f"c{i}", name=f"qps{i}")
                           for i in range(QH + KVH)]
                    for k in range(KC):
                        xk = xc[:, k * 512:(k + 1) * 512]
                        for h in range(QH):
                            nc.tensor.matmul(
                                qps[h][:], wqp[:, k * 512 + h * 128:k * 512 + (h + 1) * 128],
                                xk, start=(k == 0), stop=(k == KC - 1))
                        for g in range(KVH):
                            nc.tensor.matmul(
                                qps[QH + g][:], wkp[:, k * 256 + g * 128:k * 256 + (g + 1) * 128],
                                xk, start=(k == 0), stop=(k == KC - 1))
                    for h in range(QH):
                        drain_rope(qps[h], QT[h], nq, True)
                    for g in range(KVH):
                        drain_rope(qps[QH + g], KT[g], nq, False)
                    for sub in range(4):
                        mk = nq * 4 + sub
                        ps = v_psum.tile([128, KVH * 128], F32, tag="vps")
                        for k in range(KC):
                            nc.tensor.matmul(
                                ps[:], xc[:, k * 512 + sub * 128:k * 512 + (sub + 1) * 128],
                                wvp[:, k * 256:(k + 1) * 256],
                                start=(k == 0), stop=(k == KC - 1))
                        for g in range(KVH):
                            nc.vector.tensor_copy(
                                VE[g][:, mk * 129:mk * 129 + 128],
                                ps[:, g * 128:(g + 1) * 128])
                            nc.vector.memset(
                                VE[g][:, mk * 129 + 128:mk * 129 + 129], 1.0)

            # ---------------- phase 2: attention + o_proj ----------------
            # ACT (tanh+exp over all score tiles, ~170us) nearly matches the
            # PE work (~240us), so the PE instruction stream must interleave
            # ACT-gated score matmuls with ACT-independent filler (PV chains,
            # transposes, o_proj of the previous row-chunk) at ~2us
            # granularity — the PE executes in order, and a score matmul
            # whose PSUM slot is still being read by tanh/exp blocks
            # everything behind it.
            from collections import deque

            with (
                tc.tile_pool(name="wo", bufs=1) as wop,
                tc.tile_pool(name="pt", bufs=2) as ptp,
                tc.tile_pool(name="attnt", bufs=2) as attp,
                tc.tile_pool(name="small", bufs=3) as small,
                tc.tile_pool(name="ostage", bufs=4) as ostage,
                tc.tile_pool(name="sc_psum", bufs=2, space="PSUM") as sc_psum,
                tc.tile_pool(name="misc_psum", bufs=2, space="PSUM") as misc_psum,
            ):
                wot = wop.tile([128, QH * D], BF16, tag="wop", name="wot")
                for h in range(QH):
                    nc.sync.dma_start(wot[:, h * D:(h + 1) * D], wop_d[:, h * D:(h + 1) * D])
                WO = [wot[:, h * D:(h + 1) * D] for h in range(QH)]

                filler = deque()
                debt = [0]

                def emit_filler(budget):
                    # debt carries over so total filler consumed tracks total
                    # budget — no early queue drain, no starvation at the tail
                    debt[0] += budget
                    while filler and debt[0] > 0:
                        cost, fn = filler.popleft()
                        fn()
                        debt[0] -= cost

                def pv_unit(nq, h, g, pts, attn_qs, s):
                    def emit():
                        nks = 4 * nq + s + 1
                        pv = misc_psum.tile([128, 129], F32, tag="pv", name="pv")
                        for mk in range(nks):
                            j, hf = mk // 2, mk % 2
                            nc.tensor.matmul(
                                pv[:], pts[j][:, hf * 512 + s * 128:hf * 512 + (s + 1) * 128],
                                VE[g][:, mk * 129:(mk + 1) * 129],
                                start=(mk == 0), stop=(mk == nks - 1))
                        recip = small.tile([128, 1], F32, tag="recip", name="recip")
                        nc.vector.reciprocal(recip[:], pv[:, 128:129])
                        attn_q = small.tile([128, 128], BF16, tag="attnq", name="attnq",
                                            bufs=5)
                        nc.vector.tensor_scalar_mul(attn_q[:], pv[:, 0:128], recip[:])
                        attn_qs.append(attn_q)
                    return ((4 * nq + s + 1) * 85 + 450, emit)

                def tr_unit(attnT_h, attn_qs):
                    # all 4 transposes of a head batched, deferred a unit or
                    # more behind the DVE muls that feed them — back-to-back
                    # PE transposes pipeline instead of stalling one by one
                    def emit():
                        for s in range(4):
                            tp = misc_psum.tile([128, 128], BF16, tag="pv", name="tp")
                            nc.tensor.transpose(tp[:], attn_qs[s][:], ident[:])
                            nc.vector.tensor_copy(attnT_h[:, s * 128:(s + 1) * 128], tp[:])
                    return (1300, emit)

                def oproj_unit(nq, s, j0, j1, attnT, ob):
                    # a run of 3 consecutive 512-wide psum chunks keeps the
                    # N=512 matmuls back-to-back (LDWEIGHTS stays hidden)
                    def emit():
                        for j in range(j0, j1):
                            po = misc_psum.tile([128, 512], F32, tag="op", name="po")
                            for h in range(QH):
                                nc.tensor.matmul(
                                    po[:], attnT[h][:, s * 128:(s + 1) * 128],
                                    WO[h][:, j * 512:(j + 1) * 512],
                                    start=(h == 0), stop=(h == QH - 1))
                            nc.vector.tensor_copy(ob[:, j * 512:(j + 1) * 512], po[:])
                        if j1 == DOUT_CHUNKS:
                            row = nq * 512 + s * 128
                            nc.sync.dma_start(out_d[row:row + 128, :], ob[:])
                    return ((j1 - j0) * 4 * 216 + 100, emit)

                for nq in range(NQ):
                    # keep some filler in reserve in the last round so the
                    # flush tail (PV of the last head) has PE work to overlap
                    # the trailing exp latency
                    bf = 0.88 if nq == NQ - 1 else 1.0
                    attnT = [attp.tile([128, 512], BF16, tag=f"at{h}", name=f"at{h}")
                             for h in range(QH)]
                    for h in range(QH):
                        g = h // 2
                        pts = []
                        # full-tile pairs: two 512-wide score matmuls into one
                        # 2-bank psum slot, tanh'd in place and exp'd as a
                        # single [128,1024] activation each
                        for j in range(2 * nq):
                            ps = sc_psum.tile([128, 1024], F32, tag="s", name="scp")
                            for hf in range(2):
                                mk = 2 * j + hf
                                nc.tensor.matmul(
                                    ps[:, hf * 512:(hf + 1) * 512],
                                    KT[g][:, mk * 128:(mk + 1) * 128],
                                    QT[h][:, nq * 512:(nq + 1) * 512])
                            nc.scalar.activation(ps[:], ps[:], AF.Tanh, scale=SCALE / SOFTCAP)
                            pt = ptp.tile([128, 1024], BF16, tag=f"p{j}", name=f"pt{j}")
                            nc.scalar.activation(pt[:], ps[:], AF.Exp, scale=SOFTCAP)
                            pts.append(pt)
                            emit_filler(int(1700 * bf))
                        # diagonal tiles: per-tile activations at their causal
                        # widths, shared psum slot + pt tile per pair
                        ps = pt = None
                        for o in range(4):
                            mk = 4 * nq + o
                            j, hf = mk // 2, mk % 2
                            c0 = o * 128
                            w = 512 - c0
                            if hf == 0 or ps is None:
                                ps = sc_psum.tile([128, 1024], F32, tag="s", name="scp")
                                pt = ptp.tile([128, 1024], BF16, tag=f"p{j}", name=f"pt{j}")
                                pts.append(pt)
                            nc.tensor.matmul(
                                ps[:, hf * 512:hf * 512 + w],
                                KT[g][:, mk * 128:(mk + 1) * 128],
                                QT[h][:, nq * 512 + c0:(nq + 1) * 512])
                            nc.scalar.activation(
                                ps[:, hf * 512:hf * 512 + w], ps[:, hf * 512:hf * 512 + w],
                                AF.Tanh, scale=SCALE / SOFTCAP)
                            nc.scalar.activation(
                                pt[:, hf * 512 + c0:hf * 512 + 512],
                                ps[:, hf * 512:hf * 512 + w], AF.Exp, scale=SOFTCAP)
                            nc.vector.tensor_mul(
                                pt[:, hf * 512 + c0:hf * 512 + 512],
                                pt[:, hf * 512 + c0:hf * 512 + 512], maskt[o][:, c0:512])
                            emit_filler(int(bf * 1.9 * (w * 0.833 + 250)) - int(w / 2.4))
                        attn_qs = []
                        for s in range(4):
                            filler.append(pv_unit(nq, h, g, pts, attn_qs, s))
                        filler.append(tr_unit(attnT[h], attn_qs))
                    for s in range(4):
                        ob = ostage.tile([128, D], BF16, tag="ob", name="ob")
                        for j0 in range(0, DOUT_CHUNKS, 3):
                            filler.append(oproj_unit(nq, s, j0, min(j0 + 3, DOUT_CHUNKS), attnT, ob))
                # drain remaining PV + o_proj work (pure PE tail, ACT idle)
                while filler:
                    filler.popleft()[1]()


_CACHED_NC = {}


def build(n_iters=1):
    if n_iters not in _CACHED_NC:
        nc = bacc.Bacc("TRN2", target_bir_lowering=False, debug=False)
        _emit(nc, n_iters)
        nc.compile()
        _CACHED_NC[n_iters] = nc
    return _CACHED_NC[n_iters]


def host_tables():
    inv_freq = 1.0 / (ROPE_THETA ** (np.arange(0, HEAD_DIM, 2, dtype=np.float32) / HEAD_DIM))
    ang = np.arange(L, dtype=np.float32)[:, None] * inv_freq[None, :]  # [L, 64]
    cos, sin = np.cos(ang), np.sin(ang)
    cosT = np.concatenate([cos.T, cos.T], axis=0).astype(BF16_NP)
    sinT = np.concatenate([-sin.T, sin.T], axis=0).astype(BF16_NP)
    return np.ascontiguousarray(cosT), np.ascontiguousarray(sinT)


def host_masks():
    k = np.arange(128)[:, None]
    q = np.arange(512)[None, :]
    m = np.stack([(q >= k + 128 * o) for o in range(4)]).astype(BF16_NP)
    return np.ascontiguousarray(m)


def _pack_rows(a, width):
    """[KC*128, width] -> [128, KC*width]: chunk k's 128 rows become the
    partition dim, chunks concatenated along the free dim."""
    kc = a.shape[0] // 128
    return np.ascontiguousarray(
        a.reshape(kc, 128, width).transpose(1, 0, 2).reshape(128, kc * width))


def make_in_maps(x, wq, wk, wv, wo):
    cosT, sinT = host_tables()
    masks = host_masks()
    xt = x.reshape(L, D).T.astype(BF16_NP)          # [D, L]
    # xp[nq][p, k*512+c] = xt[k*128+p, nq*512+c]
    xp = np.ascontiguousarray(
        xt.reshape(KC, 128, NQ, 512).transpose(2, 1, 0, 3).reshape(NQ, 128, KC * 512))
    in_maps = []
    for c in range(N_CORES):
        qs = slice(c * QH * 128, (c + 1) * QH * 128)
        kvs = slice(c * KVH * 128, (c + 1) * KVH * 128)
        in_maps.append({
            "xp": xp,
            "wqp": _pack_rows(wq[qs].T.astype(BF16_NP), QH * 128),
            "wkp": _pack_rows(wk[kvs].T.astype(BF16_NP), KVH * 128),
            "wvp": _pack_rows(wv[kvs].T.astype(BF16_NP), KVH * 128),
            "wop": _pack_rows(wo[:, qs].T.astype(BF16_NP), D),
            "cost": cosT,
            "sint": sinT,
            "masks": masks,
        })
    return in_maps


def run(inputs, trace=False, trace_kwargs=None):
    from concourse.bass_utils import run_bass_kernel_spmd

    nc = build()
    x = np.asarray(inputs["x"], dtype=np.float32)
    in_maps = make_in_maps(
        x,
        np.asarray(inputs["wq"], dtype=np.float32),
        np.asarray(inputs["wk"], dtype=np.float32),
        np.asarray(inputs["wv"], dtype=np.float32),
        np.asarray(inputs["wo"], dtype=np.float32),
    )
    res = run_bass_kernel_spmd(
        nc, in_maps, core_ids=list(range(N_CORES)),
        trace=trace, **(trace_kwargs or {}))
    out = np.zeros((L, D), dtype=np.float32)
    for c in range(N_CORES):
        out += np.asarray(res.results[c]["out"], dtype=np.float32)
    return out.reshape(x.shape), res


def kernel(**inputs) -> np.ndarray:
    out, _ = run(inputs, trace=False)
    return out



# revision 38
# speedup vs baseline: 1.0153x; 1.0153x over previous
"""Trainium2 Bass kernel for GQA attention (32 q heads / 16 kv heads, head_dim
128, L=2048, D=4608) with RoPE, tanh softcap 50, causal mask, o_proj.

Strategy: tensor-parallel over heads across 8 NeuronCores. Core c computes
q-heads 4c..4c+3 and kv-heads 2c..2c+1 end-to-end (QKV projections, RoPE,
softcapped causal attention, and the partial o_proj against its 512 columns of
wo); the host sums the 8 partial [L, D] outputs.

Per-core layout choices:
  - host passes x, weights pre-transposed (xT [D,L], wqT/wkT/wvT [D,*],
    woT [512,D]) and bf16-cast; all matmuls run bf16 with fp32 PSUM accumulation
  - projections produce QT/KT/VT in [head_dim, L] layout directly
  - RoPE applied in [d, l] layout via host cos/sin tables ([128, L]; sin table
    carries the rotate-half sign split); the 1/sqrt(144) q-scale is folded into
    the softcap activation scale
  - scores computed transposed, ST[k, q] = (KT tile)^T @ QT chunk, so the
    exp(softcap) output PT[k, q] feeds the PV matmul as lhsT with no transpose
  - softcap bounds scores to +-50 so softmax needs no max-subtraction:
    p = exp(50*tanh(s/600)), denominator = sum_k p obtained free via a
    ones-column appended to the V rhs tile
  - attention output [q, d] is divided by the denominator, PE-transposed to
    [d, q], and accumulated into the o_proj over the 4 local heads
"""

import os

import numpy as np
import ml_dtypes

import concourse.bass as bass
import concourse.mybir as mybir
import concourse.tile as tile
from concourse.masks import make_identity
from concourse import bacc

F32 = mybir.dt.float32
BF16 = mybir.dt.bfloat16
BF16_NP = ml_dtypes.bfloat16
AF = mybir.ActivationFunctionType

N_HEADS = 32
N_KV = 16
HEAD_DIM = 128
ROPE_THETA = 10000.0
SOFTCAP = 50.0
SCALE = 1.0 / 12.0  # 1/sqrt(144)
L = 2048
D = 4608
N_CORES = 8
QH = N_HEADS // N_CORES        # 4 local q heads
KVH = N_KV // N_CORES          # 2 local kv heads
KC = D // 128                  # 36 contraction chunks
NQ = L // 512                  # 4 l-chunks of 512
LT = L // 128                  # 16 l-tiles of 128
DOUT_CHUNKS = D // 512         # 9 o_proj output chunks


def _emit(nc, n_iters=1):
    # packed layouts: k-chunks concatenated along the free dim so DMA lines
    # are multi-KB and descriptor counts are small (the DMA fabric moves
    # ~1KB-line tiles at only ~165 GB/s, descriptor issue ~0.6us each)
    xp_d = nc.dram_tensor("xp", [NQ, 128, KC * 512], BF16, kind="ExternalInput")
    wqp_d = nc.dram_tensor("wqp", [128, KC * 512], BF16, kind="ExternalInput")
    wkp_d = nc.dram_tensor("wkp", [128, KC * 256], BF16, kind="ExternalInput")
    wvp_d = nc.dram_tensor("wvp", [128, KC * 256], BF16, kind="ExternalInput")
    wop_d = nc.dram_tensor("wop", [128, QH * D], BF16, kind="ExternalInput")
    cost_d = nc.dram_tensor("cost", [128, L], BF16, kind="ExternalInput")
    sint_d = nc.dram_tensor("sint", [128, L], BF16, kind="ExternalInput")
    masks_d = nc.dram_tensor("masks", [4, 128, 512], BF16, kind="ExternalInput")
    out_d = nc.dram_tensor("out", [L, D], BF16, kind="ExternalOutput")

    with tile.TileContext(nc) as tc:
        with (
            tc.tile_pool(name="const", bufs=1) as const,
            tc.tile_pool(name="persist", bufs=1) as persist,
        ):
            ident = const.tile([128, 128], BF16)
            make_identity(nc, ident[:])
            cost = const.tile([128, L], BF16)
            sint = const.tile([128, L], BF16)
            maskt = [const.tile([128, 512], BF16, tag=f"mask{o}", name=f"mask{o}")
                     for o in range(4)]

            def load_consts():
                # issued after the prologue weight/x DMAs — first needed at
                # the first rope drain, ~50us in
                nc.sync.dma_start(cost[:], cost_d[:])
                nc.sync.dma_start(sint[:], sint_d[:])
                for o in range(4):
                    nc.sync.dma_start(maskt[o][:], masks_d[o])

            # persistent per-head tensors
            QT = [persist.tile([128, L], BF16, tag=f"qt{h}", name=f"qt{h}") for h in range(QH)]
            KT = [persist.tile([128, L], BF16, tag=f"kt{g}", name=f"kt{g}") for g in range(KVH)]
            # V extended with a ones column per k-tile: [128, 16*129]
            VE = [persist.tile([128, LT * 129], BF16, tag=f"ve{g}", name=f"ve{g}") for g in range(KVH)]

            def body():
                _phases(nc, tc, ident, cost, sint, maskt, QT, KT, VE,
                        xp_d, wqp_d, wkp_d, wvp_d, wop_d, out_d, load_consts)

            if n_iters == 1:
                body()
            else:
                with tc.For_i(0, n_iters, 1,
                              hint_engines=(mybir.EngineType.PE,
                                            mybir.EngineType.Activation,
                                            mybir.EngineType.DVE,
                                            mybir.EngineType.SP)):
                    body()
    return nc


def _phases(nc, tc, ident, cost, sint, maskt, QT, KT, VE,
            xp_d, wqp_d, wkp_d, wvp_d, wop_d, out_d, load_consts):
            # ---------------- phase 1: projections + rope ----------------
            def drain_rope(ps, dst, nq, fold):
                """psum [128,512] f32 -> rope -> dst bf16 [128,512] slice."""
                cols = slice(nq * 512, (nq + 1) * 512)
                raw = rtmp.tile([128, 512], F32, tag="raw")
                nc.scalar.activation(raw[:], ps[:], AF.Copy)
                swap = rtmp.tile([128, 512], F32, tag="swap")
                nc.scalar.activation(swap[0:64, :], ps[64:128, :], AF.Copy)
                nc.scalar.activation(swap[64:128, :], ps[0:64, :], AF.Copy)
                nc.vector.tensor_mul(raw[:], raw[:], cost[:, cols])
                nc.vector.tensor_mul(swap[:], swap[:], sint[:, cols])
                nc.vector.tensor_add(dst[:, cols], raw[:], swap[:])

            GK = 6           # k-chunks per DMA part
            NPART = KC // GK  # 6 parts per packed tensor
            with (
                tc.tile_pool(name="xcol", bufs=2) as xcol,
                tc.tile_pool(name="rtmp", bufs=3) as rtmp,
                tc.tile_pool(name="wts", bufs=1) as wts,
                tc.tile_pool(name="qk_psum", bufs=1, space="PSUM") as qk_psum,
                tc.tile_pool(name="v_psum", bufs=2, space="PSUM") as v_psum,
            ):
                # k-streamed projections: per 128-wide x-column chunk k, run
                # the 4 Q chains + 2 K chains (6 persistent PSUM banks), so
                # the PE paces just behind the part-granular DMA stream
                # (wq parts on the sync queue; x/wk parts on the scalar
                # queue). V (x stationary as lhsT) runs after each nq's QK
                # block, while the rope drains free the QK banks.
                wqp = wts.tile([128, KC * 512], BF16, tag="wqp", name="wqp")
                wkp = wts.tile([128, KC * 256], BF16, tag="wkp", name="wkp")
                wvp = wts.tile([128, KC * 256], BF16, tag="wvp", name="wvp")
                xc0 = xcol.tile([128, KC * 512], BF16, tag="xp", name="xc0")
                bounds = [0, 1, 2, 4, 6, 12, 18, 24, 30, 36]  # small parts first
                for k0, k1 in zip(bounds, bounds[1:]):
                    q5 = slice(k0 * 512, k1 * 512)
                    q2 = slice(k0 * 256, k1 * 256)
                    nc.sync.dma_start(wqp[:, q5], wqp_d[:, q5])
                    nc.scalar.dma_start(xc0[:, q5], xp_d[0][:, q5])
                    nc.scalar.dma_start(wkp[:, q2], wkp_d[:, q2])
                    nc.sync.dma_start(wvp[:, q2], wvp_d[:, q2])
                load_consts()

                for nq in range(NQ):
                    if nq == 0:
                        xc = xc0
                    else:
                        xc = xcol.tile([128, KC * 512], BF16, tag="xp", name=f"xc{nq}")
                        for p in range(NPART):
                            q5 = slice(p * GK * 512, (p + 1) * GK * 512)
                            nc.sync.dma_start(xc[:, q5], xp_d[nq][:, q5])
                    qps = [qk_psum.tile([128, 512], F32, tag=f"c{i}", name=f"qps{i}")
                           for i in range(QH + KVH)]
                    for k in range(KC):
                        xk = xc[:, k * 512:(k + 1) * 512]
                        for h in range(QH):
                            nc.tensor.matmul(
                                qps[h][:], wqp[:, k * 512 + h * 128:k * 512 + (h + 1) * 128],
                                xk, start=(k == 0), stop=(k == KC - 1))
                        for g in range(KVH):
                            nc.tensor.matmul(
                                qps[QH + g][:], wkp[:, k * 256 + g * 128:k * 256 + (g + 1) * 128],
                                xk, start=(k == 0), stop=(k == KC - 1))
                    for h in range(QH):
                        drain_rope(qps[h], QT[h], nq, True)
                    for g in range(KVH):
                        drain_rope(qps[QH + g], KT[g], nq, False)
                    for sub in range(4):
                        mk = nq * 4 + sub
                        ps = v_psum.tile([128, KVH * 128], F32, tag="vps")
                        for k in range(KC):
                            nc.tensor.matmul(
                                ps[:], xc[:, k * 512 + sub * 128:k * 512 + (sub + 1) * 128],
                                wvp[:, k * 256:(k + 1) * 256],
                                start=(k == 0), stop=(k == KC - 1))
                        for g in range(KVH):
                            nc.vector.tensor_copy(
                                VE[g][:, mk * 129:mk * 129 + 128],
                                ps[:, g * 128:(g + 1) * 128])
                            nc.vector.memset(
                                VE[g][:, mk * 129 + 128:mk * 129 + 129], 1.0)

            # ---------------- phase 2: attention + o_proj ----------------
            # ACT (tanh+exp over all score tiles, ~170us) nearly matches the
            # PE work (~240us), so the PE instruction stream must interleave
            # ACT-gated score matmuls with ACT-independent filler (PV chains,
            # transposes, o_proj of the previous row-chunk) at ~2us
            # granularity — the PE executes in order, and a score matmul
            # whose PSUM slot is still being read by tanh/exp blocks
            # everything behind it.
            from collections import deque

            with (
                tc.tile_pool(name="wo", bufs=1) as wop,
                tc.tile_pool(name="pt", bufs=2) as ptp,
                tc.tile_pool(name="attnt", bufs=2) as attp,
                tc.tile_pool(name="small", bufs=3) as small,
                tc.tile_pool(name="ostage", bufs=4) as ostage,
                tc.tile_pool(name="sc_psum", bufs=2, space="PSUM") as sc_psum,
                tc.tile_pool(name="misc_psum", bufs=2, space="PSUM") as misc_psum,
            ):
                wot = wop.tile([128, QH * D], BF16, tag="wop", name="wot")
                for h in range(QH):
                    nc.sync.dma_start(wot[:, h * D:(h + 1) * D], wop_d[:, h * D:(h + 1) * D])
                WO = [wot[:, h * D:(h + 1) * D] for h in range(QH)]

                filler = deque()
                debt = [0]

                def emit_filler(budget):
                    # debt carries over so total filler consumed tracks total
                    # budget — no early queue drain, no starvation at the tail
                    debt[0] += budget
                    while filler and debt[0] > 0:
                        cost, fn = filler.popleft()
                        fn()
                        debt[0] -= cost

                def pv_unit(nq, h, g, pts, attn_qs, s):
                    def emit():
                        nks = 4 * nq + s + 1
                        pv = misc_psum.tile([128, 129], F32, tag="pv", name="pv")
                        for mk in range(nks):
                            j, hf = mk // 2, mk % 2
                            nc.tensor.matmul(
                                pv[:], pts[j][:, hf * 512 + s * 128:hf * 512 + (s + 1) * 128],
                                VE[g][:, mk * 129:(mk + 1) * 129],
                                start=(mk == 0), stop=(mk == nks - 1))
                        recip = small.tile([128, 1], F32, tag="recip", name="recip")
                        nc.vector.reciprocal(recip[:], pv[:, 128:129])
                        attn_q = small.tile([128, 128], BF16, tag="attnq", name="attnq",
                                            bufs=5)
                        nc.vector.tensor_scalar_mul(attn_q[:], pv[:, 0:128], recip[:])
                        attn_qs.append(attn_q)
                    return ((4 * nq + s + 1) * 85 + 450, emit)

                def tr_unit(attnT_h, attn_qs):
                    # all 4 transposes of a head batched, deferred a unit or
                    # more behind the DVE muls that feed them — back-to-back
                    # PE transposes pipeline instead of stalling one by one
                    def emit():
                        for s in range(4):
                            tp = misc_psum.tile([128, 128], BF16, tag="pv", name="tp")
                            nc.tensor.transpose(tp[:], attn_qs[s][:], ident[:])
                            nc.vector.tensor_copy(attnT_h[:, s * 128:(s + 1) * 128], tp[:])
                    return (1300, emit)

                def oproj_unit(nq, s, j0, j1, attnT, ob):
                    # a run of 3 consecutive 512-wide psum chunks keeps the
                    # N=512 matmuls back-to-back (LDWEIGHTS stays hidden)
                    def emit():
                        for j in range(j0, j1):
                            po = misc_psum.tile([128, 512], F32, tag="op", name="po")
                            for h in range(QH):
                                nc.tensor.matmul(
                                    po[:], attnT[h][:, s * 128:(s + 1) * 128],
                                    WO[h][:, j * 512:(j + 1) * 512],
                                    start=(h == 0), stop=(h == QH - 1))
                            nc.vector.tensor_copy(ob[:, j * 512:(j + 1) * 512], po[:])
                        if j1 == DOUT_CHUNKS:
                            row = nq * 512 + s * 128
                            nc.sync.dma_start(out_d[row:row + 128, :], ob[:])
                    return ((j1 - j0) * 4 * 216 + 100, emit)

                for nq in range(NQ):
                    # keep some filler in reserve in the last round so the
                    # flush tail (PV of the last head) has PE work to overlap
                    # the trailing exp latency
                    bf = 0.88 if nq == NQ - 1 else 1.0
                    attnT = [attp.tile([128, 512], BF16, tag=f"at{h}", name=f"at{h}")
                             for h in range(QH)]
                    for h in range(QH):
                        g = h // 2
                        pts = []
                        # full-tile pairs: two 512-wide score matmuls into one
                        # 2-bank psum slot, tanh'd in place and exp'd as a
                        # single [128,1024] activation each
                        for j in range(2 * nq):
                            ps = sc_psum.tile([128, 1024], F32, tag="s", name="scp")
                            for hf in range(2):
                                mk = 2 * j + hf
                                nc.tensor.matmul(
                                    ps[:, hf * 512:(hf + 1) * 512],
                                    KT[g][:, mk * 128:(mk + 1) * 128],
                                    QT[h][:, nq * 512:(nq + 1) * 512])
                            nc.scalar.activation(ps[:], ps[:], AF.Tanh, scale=SCALE / SOFTCAP)
                            pt = ptp.tile([128, 1024], BF16, tag=f"p{j}", name=f"pt{j}")
                            nc.scalar.activation(pt[:], ps[:], AF.Exp, scale=SOFTCAP)
                            pts.append(pt)
                            emit_filler(int(1700 * bf))
                        # diagonal tiles: per-tile activations at their causal
                        # widths, shared psum slot + pt tile per pair
                        ps = pt = None
                        for o in range(4):
                            mk = 4 * nq + o
                            j, hf = mk // 2, mk % 2
                            c0 = o * 128
                            w = 512 - c0
                            if hf == 0 or ps is None:
                                ps = sc_psum.tile([128, 1024], F32, tag="s", name="scp")
                                pt = ptp.tile([128, 1024], BF16, tag=f"p{j}", name=f"pt{j}")
                                pts.append(pt)
                            nc.tensor.matmul(
                                ps[:, hf * 512:hf * 512 + w],
                                KT[g][:, mk * 128:(mk + 1) * 128],
                                QT[h][:, nq * 512 + c0:(nq + 1) * 512])
                            nc.scalar.activation(
                                ps[:, hf * 512:hf * 512 + w], ps[:, hf * 512:hf * 512 + w],
                                AF.Tanh, scale=SCALE / SOFTCAP)
                            nc.scalar.activation(
                                pt[:, hf * 512 + c0:hf * 512 + 512],
                                ps[:, hf * 512:hf * 512 + w], AF.Exp, scale=SOFTCAP)
                            nc.vector.tensor_mul(
                                pt[:, hf * 512 + c0:hf * 512 + 512],
                                pt[:, hf * 512 + c0:hf * 512 + 512], maskt[o][:, c0:512])
                            emit_filler(int(bf * 1.9 * (w * 0.833 + 250)) - int(w / 2.4))
                        attn_qs = []
                        for s in range(4):
                            filler.append(pv_unit(nq, h, g, pts, attn_qs, s))
                        filler.append(tr_unit(attnT[h], attn_qs))
                    for s in range(4):
                        ob = ostage.tile([128, D], BF16, tag="ob", name="ob")
                        for j0 in range(0, DOUT_CHUNKS, 3):
                            filler.append(oproj_unit(nq, s, j0, min(j0 + 3, DOUT_CHUNKS), attnT, ob))
                # drain remaining PV + o_proj work (pure PE tail, ACT idle)
                while filler:
                    filler.popleft()[1]()


_CACHED_NC = {}


def build(n_iters=1):
    if n_iters not in _CACHED_NC:
        nc = bacc.Bacc("TRN2", target_bir_lowering=False, debug=False)
        _emit(nc, n_iters)
        nc.compile()
        _CACHED_NC[n_iters] = nc
    return _CACHED_NC[n_iters]


def host_tables():
    inv_freq = 1.0 / (ROPE_THETA ** (np.arange(0, HEAD_DIM, 2, dtype=np.float32) / HEAD_DIM))
    ang = np.arange(L, dtype=np.float32)[:, None] * inv_freq[None, :]  # [L, 64]
    cos, sin = np.cos(ang), np.sin(ang)
    cosT = np.concatenate([cos.T, cos.T], axis=0).astype(BF16_NP)
    sinT = np.concatenate([-sin.T, sin.T], axis=0).astype(BF16_NP)
    return np.ascontiguousarray(cosT), np.ascontiguousarray(sinT)


def host_masks():
    k = np.arange(128)[:, None]
    q = np.arange(512)[None, :]
    m = np.stack([(q >= k + 128 * o) for o in range(4)]).astype(BF16_NP)
    return np.ascontiguousarray(m)


def _pack_rows(a, width):
    """[KC*128, width] -> [128, KC*width]: chunk k's 128 rows become the
    partition dim, chunks concatenated along the free dim."""
    kc = a.shape[0] // 128
    return np.ascontiguousarray(
        a.reshape(kc, 128, width).transpose(1, 0, 2).reshape(128, kc * width))


def make_in_maps(x, wq, wk, wv, wo):
    cosT, sinT = host_tables()
    masks = host_masks()
    xt = x.reshape(L, D).T.astype(BF16_NP)          # [D, L]
    # xp[nq][p, k*512+c] = xt[k*128+p, nq*512+c]
    xp = np.ascontiguousarray(
        xt.reshape(KC, 128, NQ, 512).transpose(2, 1, 0, 3).reshape(NQ, 128, KC * 512))
    in_maps = []
    for c in range(N_CORES):
        qs = slice(c * QH * 128, (c + 1) * QH * 128)
        kvs = slice(c * KVH * 128, (c + 1) * KVH * 128)
        in_maps.append({
            "xp": xp,
            "wqp": _pack_rows(wq[qs].T.astype(BF16_NP), QH * 128),
            "wkp": _pack_rows(wk[kvs].T.astype(BF16_NP), KVH * 128),
            "wvp": _pack_rows(wv[kvs].T.astype(BF16_NP), KVH * 128),
            "wop": _pack_rows(wo[:, qs].T.astype(BF16_NP), D),
            "cost": cosT,
            "sint": sinT,
            "masks": masks,
        })
    return in_maps


def run(inputs, trace=False, trace_kwargs=None):
    from concourse.bass_utils import run_bass_kernel_spmd

    nc = build()
    x = np.asarray(inputs["x"], dtype=np.float32)
    in_maps = make_in_maps(
        x,
        np.asarray(inputs["wq"], dtype=np.float32),
        np.asarray(inputs["wk"], dtype=np.float32),
        np.asarray(inputs["wv"], dtype=np.float32),
        np.asarray(inputs["wo"], dtype=np.float32),
    )
    res = run_bass_kernel_spmd(
        nc, in_maps, core_ids=list(range(N_CORES)),
        trace=trace, **(trace_kwargs or {}))
    out = np.zeros((L, D), dtype=np.float32)
    for c in range(N_CORES):
        out += np.asarray(res.results[c]["out"], dtype=np.float32)
    return out.reshape(x.shape), res


def kernel(**inputs) -> np.ndarray:
    out, _ = run(inputs, trace=False)
    return out



# revision 39
# speedup vs baseline: 1.0197x; 1.0044x over previous
"""Trainium2 Bass kernel for GQA attention (32 q heads / 16 kv heads, head_dim
128, L=2048, D=4608) with RoPE, tanh softcap 50, causal mask, o_proj.

Strategy: tensor-parallel over heads across 8 NeuronCores. Core c computes
q-heads 4c..4c+3 and kv-heads 2c..2c+1 end-to-end (QKV projections, RoPE,
softcapped causal attention, and the partial o_proj against its 512 columns of
wo); the host sums the 8 partial [L, D] outputs.

Per-core layout choices:
  - host passes x, weights pre-transposed (xT [D,L], wqT/wkT/wvT [D,*],
    woT [512,D]) and bf16-cast; all matmuls run bf16 with fp32 PSUM accumulation
  - projections produce QT/KT/VT in [head_dim, L] layout directly
  - RoPE applied in [d, l] layout via host cos/sin tables ([128, L]; sin table
    carries the rotate-half sign split); the 1/sqrt(144) q-scale is folded into
    the softcap activation scale
  - scores computed transposed, ST[k, q] = (KT tile)^T @ QT chunk, so the
    exp(softcap) output PT[k, q] feeds the PV matmul as lhsT with no transpose
  - softcap bounds scores to +-50 so softmax needs no max-subtraction:
    p = exp(50*tanh(s/600)), denominator = sum_k p obtained free via a
    ones-column appended to the V rhs tile
  - attention output [q, d] is divided by the denominator, PE-transposed to
    [d, q], and accumulated into the o_proj over the 4 local heads
"""

import os

import numpy as np
import ml_dtypes

import concourse.bass as bass
import concourse.mybir as mybir
import concourse.tile as tile
from concourse.masks import make_identity
from concourse import bacc

F32 = mybir.dt.float32
BF16 = mybir.dt.bfloat16
BF16_NP = ml_dtypes.bfloat16
AF = mybir.ActivationFunctionType

N_HEADS = 32
N_KV = 16
HEAD_DIM = 128
ROPE_THETA = 10000.0
SOFTCAP = 50.0
SCALE = 1.0 / 12.0  # 1/sqrt(144)
L = 2048
D = 4608
N_CORES = 8
QH = N_HEADS // N_CORES        # 4 local q heads
KVH = N_KV // N_CORES          # 2 local kv heads
KC = D // 128                  # 36 contraction chunks
NQ = L // 512                  # 4 l-chunks of 512
LT = L // 128                  # 16 l-tiles of 128
DOUT_CHUNKS = D // 512         # 9 o_proj output chunks


def _emit(nc, n_iters=1):
    # packed layouts: k-chunks concatenated along the free dim so DMA lines
    # are multi-KB and descriptor counts are small (the DMA fabric moves
    # ~1KB-line tiles at only ~165 GB/s, descriptor issue ~0.6us each)
    xp_d = nc.dram_tensor("xp", [NQ, 128, KC * 512], BF16, kind="ExternalInput")
    wqp_d = nc.dram_tensor("wqp", [128, KC * 512], BF16, kind="ExternalInput")
    wkp_d = nc.dram_tensor("wkp", [128, KC * 256], BF16, kind="ExternalInput")
    wvp_d = nc.dram_tensor("wvp", [128, KC * 256], BF16, kind="ExternalInput")
    wop_d = nc.dram_tensor("wop", [128, QH * D], BF16, kind="ExternalInput")
    cost_d = nc.dram_tensor("cost", [128, L], BF16, kind="ExternalInput")
    sint_d = nc.dram_tensor("sint", [128, L], BF16, kind="ExternalInput")
    masks_d = nc.dram_tensor("masks", [4, 128, 512], BF16, kind="ExternalInput")
    out_d = nc.dram_tensor("out", [L, D], BF16, kind="ExternalOutput")

    with tile.TileContext(nc) as tc:
        with (
            tc.tile_pool(name="const", bufs=1) as const,
            tc.tile_pool(name="persist", bufs=1) as persist,
        ):
            ident = const.tile([128, 128], BF16)
            make_identity(nc, ident[:])
            cost = const.tile([128, L], BF16)
            sint = const.tile([128, L], BF16)
            maskt = [const.tile([128, 512], BF16, tag=f"mask{o}", name=f"mask{o}")
                     for o in range(4)]

            def load_consts():
                # issued after the prologue weight/x DMAs — first needed at
                # the first rope drain, ~50us in
                nc.sync.dma_start(cost[:], cost_d[:])
                nc.sync.dma_start(sint[:], sint_d[:])
                for o in range(4):
                    nc.sync.dma_start(maskt[o][:], masks_d[o])

            # persistent per-head tensors
            QT = [persist.tile([128, L], BF16, tag=f"qt{h}", name=f"qt{h}") for h in range(QH)]
            KT = [persist.tile([128, L], BF16, tag=f"kt{g}", name=f"kt{g}") for g in range(KVH)]
            # V extended with a ones column per k-tile: [128, 16*129]
            VE = [persist.tile([128, LT * 129], BF16, tag=f"ve{g}", name=f"ve{g}") for g in range(KVH)]

            def body():
                _phases(nc, tc, ident, cost, sint, maskt, QT, KT, VE,
                        xp_d, wqp_d, wkp_d, wvp_d, wop_d, out_d, load_consts)

            if n_iters == 1:
                body()
            else:
                with tc.For_i(0, n_iters, 1,
                              hint_engines=(mybir.EngineType.PE,
                                            mybir.EngineType.Activation,
                                            mybir.EngineType.DVE,
                                            mybir.EngineType.SP)):
                    body()
    return nc


def _phases(nc, tc, ident, cost, sint, maskt, QT, KT, VE,
            xp_d, wqp_d, wkp_d, wvp_d, wop_d, out_d, load_consts):
            # ---------------- phase 1: projections + rope ----------------
            def drain_rope(ps, dst, nq, fold):
                """psum [128,512] f32 -> rope -> dst bf16 [128,512] slice."""
                cols = slice(nq * 512, (nq + 1) * 512)
                raw = rtmp.tile([128, 512], F32, tag="raw")
                nc.scalar.activation(raw[:], ps[:], AF.Copy)
                swap = rtmp.tile([128, 512], F32, tag="swap")
                nc.scalar.activation(swap[0:64, :], ps[64:128, :], AF.Copy)
                nc.scalar.activation(swap[64:128, :], ps[0:64, :], AF.Copy)
                nc.vector.tensor_mul(raw[:], raw[:], cost[:, cols])
                nc.vector.tensor_mul(swap[:], swap[:], sint[:, cols])
                nc.vector.tensor_add(dst[:, cols], raw[:], swap[:])

            GK = 6           # k-chunks per DMA part
            NPART = KC // GK  # 6 parts per packed tensor
            with (
                tc.tile_pool(name="xcol", bufs=2) as xcol,
                tc.tile_pool(name="rtmp", bufs=3) as rtmp,
                tc.tile_pool(name="wts", bufs=1) as wts,
                tc.tile_pool(name="qk_psum", bufs=1, space="PSUM") as qk_psum,
                tc.tile_pool(name="v_psum", bufs=2, space="PSUM") as v_psum,
            ):
                # k-streamed projections: per 128-wide x-column chunk k, run
                # the 4 Q chains + 2 K chains (6 persistent PSUM banks), so
                # the PE paces just behind the part-granular DMA stream
                # (wq parts on the sync queue; x/wk parts on the scalar
                # queue). V (x stationary as lhsT) runs after each nq's QK
                # block, while the rope drains free the QK banks.
                wqp = wts.tile([128, KC * 512], BF16, tag="wqp", name="wqp")
                wkp = wts.tile([128, KC * 256], BF16, tag="wkp", name="wkp")
                wvp = wts.tile([128, KC * 256], BF16, tag="wvp", name="wvp")
                xc0 = xcol.tile([128, KC * 512], BF16, tag="xp", name="xc0")
                bounds = [0, 1, 2, 4, 6, 12, 18, 24, 30, 36]  # small parts first
                for k0, k1 in zip(bounds, bounds[1:]):
                    q5 = slice(k0 * 512, k1 * 512)
                    q2 = slice(k0 * 256, k1 * 256)
                    nc.sync.dma_start(wqp[:, q5], wqp_d[:, q5])
                    nc.scalar.dma_start(xc0[:, q5], xp_d[0][:, q5])
                    nc.scalar.dma_start(wkp[:, q2], wkp_d[:, q2])
                    nc.sync.dma_start(wvp[:, q2], wvp_d[:, q2])
                load_consts()

                for nq in range(NQ):
                    if nq == 0:
                        xc = xc0
                    else:
                        xc = xcol.tile([128, KC * 512], BF16, tag="xp", name=f"xc{nq}")
                        for p in range(NPART):
                            q5 = slice(p * GK * 512, (p + 1) * GK * 512)
                            nc.sync.dma_start(xc[:, q5], xp_d[nq][:, q5])
                    qps = [qk_psum.tile([128, 512], F32, tag=f"c{i}", name=f"qps{i}")
                           for i in range(QH + KVH)]
                    for k in range(KC):
                        xk = xc[:, k * 512:(k + 1) * 512]
                        for h in range(QH):
                            nc.tensor.matmul(
                                qps[h][:], wqp[:, k * 512 + h * 128:k * 512 + (h + 1) * 128],
                                xk, start=(k == 0), stop=(k == KC - 1))
                        for g in range(KVH):
                            nc.tensor.matmul(
                                qps[QH + g][:], wkp[:, k * 256 + g * 128:k * 256 + (g + 1) * 128],
                                xk, start=(k == 0), stop=(k == KC - 1))
                    for h in range(QH):
                        drain_rope(qps[h], QT[h], nq, True)
                    for g in range(KVH):
                        drain_rope(qps[QH + g], KT[g], nq, False)
                    for sub in range(4):
                        mk = nq * 4 + sub
                        ps = v_psum.tile([128, KVH * 128], F32, tag="vps")
                        for k in range(KC):
                            nc.tensor.matmul(
                                ps[:], xc[:, k * 512 + sub * 128:k * 512 + (sub + 1) * 128],
                                wvp[:, k * 256:(k + 1) * 256],
                                start=(k == 0), stop=(k == KC - 1))
                        for g in range(KVH):
                            nc.vector.tensor_copy(
                                VE[g][:, mk * 129:mk * 129 + 128],
                                ps[:, g * 128:(g + 1) * 128])
                            nc.vector.memset(
                                VE[g][:, mk * 129 + 128:mk * 129 + 129], 1.0)

            # ---------------- phase 2: attention + o_proj ----------------
            # ACT (tanh+exp over all score tiles, ~170us) nearly matches the
            # PE work (~240us), so the PE instruction stream must interleave
            # ACT-gated score matmuls with ACT-independent filler (PV chains,
            # transposes, o_proj of the previous row-chunk) at ~2us
            # granularity — the PE executes in order, and a score matmul
            # whose PSUM slot is still being read by tanh/exp blocks
            # everything behind it.
            from collections import deque

            with (
                tc.tile_pool(name="wo", bufs=1) as wop,
                tc.tile_pool(name="pt", bufs=2) as ptp,
                tc.tile_pool(name="attnt", bufs=2) as attp,
                tc.tile_pool(name="small", bufs=3) as small,
                tc.tile_pool(name="ostage", bufs=4) as ostage,
                tc.tile_pool(name="sc_psum", bufs=2, space="PSUM") as sc_psum,
                tc.tile_pool(name="misc_psum", bufs=2, space="PSUM") as misc_psum,
            ):
                wot = wop.tile([128, QH * D], BF16, tag="wop", name="wot")
                for h in range(QH):
                    nc.sync.dma_start(wot[:, h * D:(h + 1) * D], wop_d[:, h * D:(h + 1) * D])
                WO = [wot[:, h * D:(h + 1) * D] for h in range(QH)]

                filler = deque()
                debt = [0]

                def emit_filler(budget):
                    # debt carries over so total filler consumed tracks total
                    # budget — no early queue drain, no starvation at the tail
                    debt[0] += budget
                    while filler and debt[0] > 0:
                        cost, fn = filler.popleft()
                        fn()
                        debt[0] -= cost

                def pv_unit(nq, h, g, pts, attn_qs, s):
                    def emit():
                        nks = 4 * nq + s + 1
                        pv = misc_psum.tile([128, 129], F32, tag="pv", name="pv")
                        for mk in range(nks):
                            j, hf = mk // 2, mk % 2
                            nc.tensor.matmul(
                                pv[:], pts[j][:, hf * 512 + s * 128:hf * 512 + (s + 1) * 128],
                                VE[g][:, mk * 129:(mk + 1) * 129],
                                start=(mk == 0), stop=(mk == nks - 1))
                        recip = small.tile([128, 1], F32, tag="recip", name="recip")
                        nc.vector.reciprocal(recip[:], pv[:, 128:129])
                        attn_q = small.tile([128, 128], BF16, tag="attnq", name="attnq",
                                            bufs=5)
                        nc.vector.tensor_scalar_mul(attn_q[:], pv[:, 0:128], recip[:])
                        attn_qs.append(attn_q)
                    return ((4 * nq + s + 1) * 85 + 450, emit)

                def tr_unit(attnT_h, attn_qs):
                    # all 4 transposes of a head batched, deferred a unit or
                    # more behind the DVE muls that feed them — back-to-back
                    # PE transposes pipeline instead of stalling one by one
                    def emit():
                        for s in range(4):
                            tp = misc_psum.tile([128, 128], BF16, tag="pv", name="tp")
                            nc.tensor.transpose(tp[:], attn_qs[s][:], ident[:])
                            nc.vector.tensor_copy(attnT_h[:, s * 128:(s + 1) * 128], tp[:])
                    return (1300, emit)

                def oproj_unit(nq, s, j0, j1, attnT, ob):
                    # a run of 3 consecutive 512-wide psum chunks keeps the
                    # N=512 matmuls back-to-back (LDWEIGHTS stays hidden)
                    def emit():
                        for j in range(j0, j1):
                            po = misc_psum.tile([128, 512], F32, tag="op", name="po")
                            for h in range(QH):
                                nc.tensor.matmul(
                                    po[:], attnT[h][:, s * 128:(s + 1) * 128],
                                    WO[h][:, j * 512:(j + 1) * 512],
                                    start=(h == 0), stop=(h == QH - 1))
                            nc.vector.tensor_copy(ob[:, j * 512:(j + 1) * 512], po[:])
                        if j1 == DOUT_CHUNKS:
                            row = nq * 512 + s * 128
                            nc.sync.dma_start(out_d[row:row + 128, :], ob[:])
                    return ((j1 - j0) * 4 * 216 + 100, emit)

                for nq in range(NQ):
                    bf = 1.0
                    attnT = [attp.tile([128, 512], BF16, tag=f"at{h}", name=f"at{h}")
                             for h in range(QH)]
                    for h in range(QH):
                        g = h // 2
                        pts = []
                        # full-tile pairs: two 512-wide score matmuls into one
                        # 2-bank psum slot, tanh'd in place and exp'd as a
                        # single [128,1024] activation each
                        for j in range(2 * nq):
                            ps = sc_psum.tile([128, 1024], F32, tag="s", name="scp")
                            for hf in range(2):
                                mk = 2 * j + hf
                                nc.tensor.matmul(
                                    ps[:, hf * 512:(hf + 1) * 512],
                                    KT[g][:, mk * 128:(mk + 1) * 128],
                                    QT[h][:, nq * 512:(nq + 1) * 512])
                            nc.scalar.activation(ps[:], ps[:], AF.Tanh, scale=SCALE / SOFTCAP)
                            pt = ptp.tile([128, 1024], BF16, tag=f"p{j}", name=f"pt{j}")
                            nc.scalar.activation(pt[:], ps[:], AF.Exp, scale=SOFTCAP)
                            pts.append(pt)
                            emit_filler(int(1700 * bf))
                        # diagonal tiles: per-tile activations at their causal
                        # widths, shared psum slot + pt tile per pair
                        ps = pt = None
                        for o in range(4):
                            mk = 4 * nq + o
                            j, hf = mk // 2, mk % 2
                            c0 = o * 128
                            w = 512 - c0
                            if hf == 0 or ps is None:
                                ps = sc_psum.tile([128, 1024], F32, tag="s", name="scp")
                                pt = ptp.tile([128, 1024], BF16, tag=f"p{j}", name=f"pt{j}")
                                pts.append(pt)
                            nc.tensor.matmul(
                                ps[:, hf * 512:hf * 512 + w],
                                KT[g][:, mk * 128:(mk + 1) * 128],
                                QT[h][:, nq * 512 + c0:(nq + 1) * 512])
                            nc.scalar.activation(
                                ps[:, hf * 512:hf * 512 + w], ps[:, hf * 512:hf * 512 + w],
                                AF.Tanh, scale=SCALE / SOFTCAP)
                            nc.scalar.activation(
                                pt[:, hf * 512 + c0:hf * 512 + 512],
                                ps[:, hf * 512:hf * 512 + w], AF.Exp, scale=SOFTCAP)
                            nc.vector.tensor_mul(
                                pt[:, hf * 512 + c0:hf * 512 + 512],
                                pt[:, hf * 512 + c0:hf * 512 + 512], maskt[o][:, c0:512])
                            emit_filler(int(bf * 1.9 * (w * 0.833 + 250)) - int(w / 2.4))
                        attn_qs = []
                        for s in range(4):
                            filler.append(pv_unit(nq, h, g, pts, attn_qs, s))
                        filler.append(tr_unit(attnT[h], attn_qs))
                    for s in range(4):
                        ob = ostage.tile([128, D], BF16, tag="ob", name="ob")
                        for j0 in range(0, DOUT_CHUNKS, 3):
                            filler.append(oproj_unit(nq, s, j0, min(j0 + 3, DOUT_CHUNKS), attnT, ob))
                # drain remaining PV + o_proj work (pure PE tail, ACT idle)
                while filler:
                    filler.popleft()[1]()


_CACHED_NC = {}


def build(n_iters=1):
    if n_iters not in _CACHED_NC:
        nc = bacc.Bacc("TRN2", target_bir_lowering=False, debug=False)
        _emit(nc, n_iters)
        nc.compile()
        _CACHED_NC[n_iters] = nc
    return _CACHED_NC[n_iters]


def host_tables():
    inv_freq = 1.0 / (ROPE_THETA ** (np.arange(0, HEAD_DIM, 2, dtype=np.float32) / HEAD_DIM))
    ang = np.arange(L, dtype=np.float32)[:, None] * inv_freq[None, :]  # [L, 64]
    cos, sin = np.cos(ang), np.sin(ang)
    cosT = np.concatenate([cos.T, cos.T], axis=0).astype(BF16_NP)
    sinT = np.concatenate([-sin.T, sin.T], axis=0).astype(BF16_NP)
    return np.ascontiguousarray(cosT), np.ascontiguousarray(sinT)


def host_masks():
    k = np.arange(128)[:, None]
    q = np.arange(512)[None, :]
    m = np.stack([(q >= k + 128 * o) for o in range(4)]).astype(BF16_NP)
    return np.ascontiguousarray(m)


def _pack_rows(a, width):
    """[KC*128, width] -> [128, KC*width]: chunk k's 128 rows become the
    partition dim, chunks concatenated along the free dim."""
    kc = a.shape[0] // 128
    return np.ascontiguousarray(
        a.reshape(kc, 128, width).transpose(1, 0, 2).reshape(128, kc * width))


def make_in_maps(x, wq, wk, wv, wo):
    cosT, sinT = host_tables()
    masks = host_masks()
    xt = x.reshape(L, D).T.astype(BF16_NP)          # [D, L]
    # xp[nq][p, k*512+c] = xt[k*128+p, nq*512+c]
    xp = np.ascontiguousarray(
        xt.reshape(KC, 128, NQ, 512).transpose(2, 1, 0, 3).reshape(NQ, 128, KC * 512))
    in_maps = []
    for c in range(N_CORES):
        qs = slice(c * QH * 128, (c + 1) * QH * 128)
        kvs = slice(c * KVH * 128, (c + 1) * KVH * 128)
        in_maps.append({
            "xp": xp,
            "wqp": _pack_rows(wq[qs].T.astype(BF16_NP), QH * 128),
            "wkp": _pack_rows(wk[kvs].T.astype(BF16_NP), KVH * 128),
            "wvp": _pack_rows(wv[kvs].T.astype(BF16_NP), KVH * 128),
            "wop": _pack_rows(wo[:, qs].T.astype(BF16_NP), D),
            "cost": cosT,
            "sint": sinT,
            "masks": masks,
        })
    return in_maps


def run(inputs, trace=False, trace_kwargs=None):
    from concourse.bass_utils import run_bass_kernel_spmd

    nc = build()
    x = np.asarray(inputs["x"], dtype=np.float32)
    in_maps = make_in_maps(
        x,
        np.asarray(inputs["wq"], dtype=np.float32),
        np.asarray(inputs["wk"], dtype=np.float32),
        np.asarray(inputs["wv"], dtype=np.float32),
        np.asarray(inputs["wo"], dtype=np.float32),
    )
    res = run_bass_kernel_spmd(
        nc, in_maps, core_ids=list(range(N_CORES)),
        trace=trace, **(trace_kwargs or {}))
    out = np.zeros((L, D), dtype=np.float32)
    for c in range(N_CORES):
        out += np.asarray(res.results[c]["out"], dtype=np.float32)
    return out.reshape(x.shape), res


def kernel(**inputs) -> np.ndarray:
    out, _ = run(inputs, trace=False)
    return out

